# revision 5
# baseline (speedup 1.0000x reference)
"""Grouped SwiGLU expert FFN (MoE) on 8 Trainium2 NeuronCores.

Expert parallelism: expert e's weights + its (pre-sorted) token slice go to
core e. Each core runs x@w1, x@w3, silu/mul, h@w2 for its 8192 tokens.

v4: fp16 datapath, PE does *pure GEMM* (zero transpose work):
 - x is cast f32->fp16 by an SWDGE (gpsimd) DMA into a DRAM bounce buffer,
   then loaded transposed straight into SBUF by the DMA xbar transpose
   (HWDGE, SP queue).  A 3D out AP [128, 8, 512] enumerates transposed rows
   c-major (i = c*128 + p), matching the standard "(c p)" weight layout
   (verified on HW by probe_xbar.py).
 - weights stream in as f32 on the ACT HWDGE queue into a staging tile and
   are cast to fp16 by DVE (2-elem/cycle copy), in column halves so mm1 can
   start ~8us in.  SP queue carries ONLY xbar transposes (no xbar-mode
   transitions); output DMAs ride the ACT queue, split per t-chunk so the
   tail overlaps mm2.
 - 512-token blocks; mm1/mm3/mm2 all 512-wide moving operands at
   1 cycle/row (fp16).

Math per core (dims: t=tokens, i=dim_in, j=dim_hid, o=dim_in):
  mm1/mm3: psum[j,t] += lhsT=w{1,3}[i_chunk, j_chunk] (stationary),
           rhs=xT[i_chunk, t_block] (moving 512) -> h1T/h3T.
  SwiGLU:  hT = silu(h1T) * h3T  (ACT Silu -> fp16, DVE mul -> fp16).
  mm2:     lhsT=hT[j_chunk, t_chunk] (stationary), rhs=w2[j_chunk, o_block]
           (moving 512) -> psum[t,o] natural-layout f32 output.
"""

import sys

sys.path.insert(0, "/opt/trn_rl_repo")

import numpy as np

N_CORES = 8
D = 1024  # dim_in
H = 1024  # dim_hid
P = 128
TB = 512  # token block per pipeline stage

_CACHE = {}


def _build(tok):
    import concourse.bacc as bacc
    import concourse.tile as tile
    from concourse import mybir

    dt = mybir.dt
    AF = mybir.ActivationFunctionType
    f32 = dt.float32
    f16 = dt.float16

    assert tok % TB == 0
    n_blk = tok // TB
    n_i = D // P   # 8 contraction chunks for mm1/mm3
    n_j = H // P   # 8 contraction chunks for mm2
    n_tc = TB // P  # 4 token chunks per block
    n_o = D // 512  # 2 output column blocks

    nc = bacc.Bacc(trn_type="TRN2", target_bir_lowering=False)
    x_h = nc.dram_tensor("x", [tok, D], f32, kind="ExternalInput")
    w1_h = nc.dram_tensor("w1", [D, H], f32, kind="ExternalInput")
    w2_h = nc.dram_tensor("w2", [H, D], f32, kind="ExternalInput")
    w3_h = nc.dram_tensor("w3", [D, H], f32, kind="ExternalInput")
    out_h = nc.dram_tensor("out", [tok, D], f32, kind="ExternalOutput")
    # DRAM bounce buffer for the fp16 copy of x (input to the xbar transpose)
    xf_h = nc.dram_tensor("xf16s", [tok, D], f16, kind="Internal")

    with tile.TileContext(nc) as tc:
        with (
            tc.tile_pool(name="wpool", bufs=1) as wpool,
            tc.tile_pool(name="wstage", bufs=2) as wst,
            tc.tile_pool(name="xtpool", bufs=3) as xtpool,
            tc.tile_pool(name="htpool", bufs=2) as htpool,
            tc.tile_pool(name="spool", bufs=3) as spool,
            tc.tile_pool(name="opool", bufs=2) as opool,
            tc.tile_pool(name="pAB", bufs=4, space="PSUM") as pABp,
            tc.tile_pool(name="pC", bufs=4, space="PSUM") as pCp,
        ):
            # ---- x: cast to fp16 in DRAM (SWDGE), blocks pipelined.
            for b in range(min(2, n_blk)):
                nc.gpsimd.dma_start(
                    out=xf_h[b * TB:(b + 1) * TB, :],
                    in_=x_h[b * TB:(b + 1) * TB, :],
                )

            # ---- weights: f32 on ACT HWDGE queue -> DVE cast to fp16,
            # in column halves so early matmuls aren't gated on full loads.
            w1s = wpool.tile([P, n_i, H], f16)
            w3s = wpool.tile([P, n_i, H], f16)
            w2s = wpool.tile([P, n_j, D], f16)
            for (wsb, wh) in ((w1s, w1_h), (w3s, w3_h), (w2s, w2_h)):
                for hhalf in range(2):
                    cs = slice(hhalf * 512, (hhalf + 1) * 512)
                    stg = wst.tile([P, 8, 512], f32, tag="wst")
                    nc.scalar.dma_start(
                        out=stg,
                        in_=wh[:, cs].rearrange("(c p) h -> p c h", p=P),
                    )
                    nc.vector.tensor_copy(wsb[:, :, cs], stg)

            o_r = out_h[:, :].rearrange("(b c p) d -> b p c d", p=P, c=n_tc)

            for b in range(n_blk):
                # cast-DMA for a later block keeps the SWDGE pipeline ahead
                nb = b + 2
                if 2 <= nb < n_blk:
                    nc.gpsimd.dma_start(
                        out=xf_h[nb * TB:(nb + 1) * TB, :],
                        in_=x_h[nb * TB:(nb + 1) * TB, :],
                    )

                # ---- xbar-transpose load: xT[p, c, t] = x[t, c*128+p]
                xT = xtpool.tile([P, n_i, TB], f16)
                nc.sync.dma_start(
                    out=xT, in_=xf_h[b * TB:(b + 1) * TB, :], transpose=True
                )

                # ---- mm1/mm3 + SwiGLU -> hT [P(=j in chunk), n_j, TB] fp16
                hT = htpool.tile([P, n_j, TB], f16)
                for j in range(n_j):
                    pA = pABp.tile([P, TB], f32, tag="pAB")
                    pB = pABp.tile([P, TB], f32, tag="pAB")
                    for i in range(n_i):
                        nc.tensor.matmul(
                            pA, w1s[:, i, j * P:(j + 1) * P], xT[:, i, :],
                            start=(i == 0), stop=(i == n_i - 1),
                        )
                    for i in range(n_i):
                        nc.tensor.matmul(
                            pB, w3s[:, i, j * P:(j + 1) * P], xT[:, i, :],
                            start=(i == 0), stop=(i == n_i - 1),
                        )
                    s1 = spool.tile([P, TB], f16)
                    nc.scalar.activation(s1, pA, AF.Silu)
                    nc.vector.tensor_mul(hT[:, j, :], pB, s1)

                # ---- mm2 -> natural-layout out block; DMA per t-chunk so
                # the last block's store overlaps its own mm2.
                o_sb = opool.tile([P, n_tc, D], f32)
                for t in range(n_tc):
                    for o in range(n_o):
                        pC = pCp.tile([P, 512], f32)
                        for j in range(n_j):
                            nc.tensor.matmul(
                                pC,
                                hT[:, j, t * P:(t + 1) * P],
                                w2s[:, j, o * 512:(o + 1) * 512],
                                start=(j == 0), stop=(j == n_j - 1),
                            )
                        nc.scalar.activation(
                            o_sb[:, t, o * 512:(o + 1) * 512], pC, AF.Copy
                        )
                    nc.scalar.dma_start(
                        out=o_r[b, :, t, :], in_=o_sb[:, t, :]
                    )

    nc.compile()
    return nc


def _get_nc(tok):
    if tok not in _CACHE:
        _CACHE[tok] = _build(tok)
    return _CACHE[tok]


def kernel(x, w1, w2, w3, m_sizes):
    from concourse.bass_utils import run_bass_kernel_spmd

    x = np.asarray(x, dtype=np.float32)
    w1 = np.asarray(w1, dtype=np.float32)
    w2 = np.asarray(w2, dtype=np.float32)
    w3 = np.asarray(w3, dtype=np.float32)
    sizes = np.asarray(m_sizes).astype(np.int64)
    offs = np.concatenate([[0], np.cumsum(sizes)])
    n_exp = sizes.shape[0]
    assert n_exp == N_CORES

    pad = int(max(int(sizes.max()), TB))
    pad = ((pad + TB - 1) // TB) * TB
    nc = _get_nc(pad)

    in_maps = []
    for e in range(N_CORES):
        xe = x[offs[e]:offs[e + 1]]
        if xe.shape[0] < pad:
            xe = np.concatenate(
                [xe, np.zeros((pad - xe.shape[0], D), dtype=np.float32)], axis=0
            )
        in_maps.append({"x": xe, "w1": w1[e], "w2": w2[e], "w3": w3[e]})

    r = run_bass_kernel_spmd(nc, in_maps, core_ids=list(range(N_CORES)))
    out = np.concatenate(
        [r.results[e]["out"][: sizes[e]] for e in range(N_CORES)], axis=0
    )
    return out.astype(np.float32)


# revision 9
# speedup vs baseline: 1.0075x; 1.0075x over previous
"""Grouped SwiGLU expert FFN (MoE) on 8 Trainium2 NeuronCores.

Expert parallelism: expert e's weights + its (pre-sorted) token slice go to
core e. Each core runs x@w1, x@w3, silu/mul, h@w2 for its 8192 tokens.

v4: fp16 datapath, PE does *pure GEMM* (zero transpose work):
 - x is cast f32->fp16 by an SWDGE (gpsimd) DMA into a DRAM bounce buffer,
   then loaded transposed straight into SBUF by the DMA xbar transpose
   (HWDGE, SP queue).  A 3D out AP [128, 8, 512] enumerates transposed rows
   c-major (i = c*128 + p), matching the standard "(c p)" weight layout
   (verified on HW by probe_xbar.py).
 - weights stream in as f32 on the ACT HWDGE queue into a staging tile and
   are cast to fp16 by DVE (2-elem/cycle copy), in column halves so mm1 can
   start ~8us in.  SP queue carries ONLY xbar transposes (no xbar-mode
   transitions); output DMAs ride the ACT queue, split per t-chunk so the
   tail overlaps mm2.
 - 512-token blocks; mm1/mm3/mm2 all 512-wide moving operands at
   1 cycle/row (fp16).

Math per core (dims: t=tokens, i=dim_in, j=dim_hid, o=dim_in):
  mm1/mm3: psum[j,t] += lhsT=w{1,3}[i_chunk, j_chunk] (stationary),
           rhs=xT[i_chunk, t_block] (moving 512) -> h1T/h3T.
  SwiGLU:  hT = silu(h1T) * h3T  (ACT Silu -> fp16, DVE mul -> fp16).
  mm2:     lhsT=hT[j_chunk, t_chunk] (stationary), rhs=w2[j_chunk, o_block]
           (moving 512) -> psum[t,o] natural-layout f32 output.
"""

import sys

sys.path.insert(0, "/opt/trn_rl_repo")

import numpy as np

N_CORES = 8
D = 1024  # dim_in
H = 1024  # dim_hid
P = 128
TB = 512  # token block per pipeline stage

_CACHE = {}


def _build(tok):
    import concourse.bacc as bacc
    import concourse.tile as tile
    from concourse import mybir

    dt = mybir.dt
    AF = mybir.ActivationFunctionType
    f32 = dt.float32
    f16 = dt.float16

    assert tok % TB == 0
    n_blk = tok // TB
    n_i = D // P   # 8 contraction chunks for mm1/mm3
    n_j = H // P   # 8 contraction chunks for mm2
    n_tc = TB // P  # 4 token chunks per block
    n_o = D // 512  # 2 output column blocks

    nc = bacc.Bacc(trn_type="TRN2", target_bir_lowering=False)
    x_h = nc.dram_tensor("x", [tok, D], f32, kind="ExternalInput")
    w1_h = nc.dram_tensor("w1", [D, H], f32, kind="ExternalInput")
    w2_h = nc.dram_tensor("w2", [H, D], f32, kind="ExternalInput")
    w3_h = nc.dram_tensor("w3", [D, H], f32, kind="ExternalInput")
    out_h = nc.dram_tensor("out", [tok, D], f32, kind="ExternalOutput")
    # DRAM bounce buffers for the fp16 copy of x (input to the xbar
    # transpose).  One tensor PER BLOCK: a single shared tensor makes Tile's
    # coarse whole-tensor dependency tracking serialize cast(b+1) behind
    # transpose(b) (measured: 41us/block serial chain).
    xf_hs = [
        nc.dram_tensor(f"xf16s_{b}", [TB, D], f16, kind="Internal")
        for b in range(tok // TB)
    ]

    with tile.TileContext(nc) as tc:
        with (
            tc.tile_pool(name="wpool", bufs=1) as wpool,
            tc.tile_pool(name="wstage", bufs=2) as wst,
            tc.tile_pool(name="xtpool", bufs=4) as xtpool,
            tc.tile_pool(name="htpool", bufs=2) as htpool,
            tc.tile_pool(name="spool", bufs=3) as spool,
            tc.tile_pool(name="opool", bufs=2) as opool,
            tc.tile_pool(name="pAB", bufs=4, space="PSUM") as pABp,
            tc.tile_pool(name="pC", bufs=4, space="PSUM") as pCp,
        ):
            # ---- x: cast to fp16 in DRAM (SWDGE), blocks pipelined.
            for b in range(min(4, n_blk)):
                nc.gpsimd.dma_start(
                    out=xf_hs[b][:, :],
                    in_=x_h[b * TB:(b + 1) * TB, :],
                )

            # ---- weights: f32 on ACT HWDGE queue -> DVE cast to fp16,
            # in column halves so early matmuls aren't gated on full loads.
            w1s = wpool.tile([P, n_i, H], f16)
            w3s = wpool.tile([P, n_i, H], f16)
            w2s = wpool.tile([P, n_j, D], f16)
            for (wsb, wh) in ((w1s, w1_h), (w3s, w3_h), (w2s, w2_h)):
                for hhalf in range(2):
                    cs = slice(hhalf * 512, (hhalf + 1) * 512)
                    stg = wst.tile([P, 8, 512], f32, tag="wst")
                    nc.scalar.dma_start(
                        out=stg,
                        in_=wh[:, cs].rearrange("(c p) h -> p c h", p=P),
                    )
                    nc.vector.tensor_copy(wsb[:, :, cs], stg)

            o_r = out_h[:, :].rearrange("(b c p) d -> b p c d", p=P, c=n_tc)

            for b in range(n_blk):
                # cast-DMA for a later block keeps the SWDGE pipeline ahead
                nb = b + 4
                if 4 <= nb < n_blk:
                    nc.gpsimd.dma_start(
                        out=xf_hs[nb][:, :],
                        in_=x_h[nb * TB:(nb + 1) * TB, :],
                    )

                # ---- xbar-transpose load: xT[p, c, t] = x[t, c*128+p]
                xT = xtpool.tile([P, n_i, TB], f16)
                nc.sync.dma_start(
                    out=xT, in_=xf_hs[b][:, :], transpose=True
                )

                # ---- mm1/mm3 + SwiGLU -> hT [P(=j in chunk), n_j, TB] fp16
                hT = htpool.tile([P, n_j, TB], f16)
                for j in range(n_j):
                    pA = pABp.tile([P, TB], f32, tag="pAB")
                    pB = pABp.tile([P, TB], f32, tag="pAB")
                    for i in range(n_i):
                        nc.tensor.matmul(
                            pA, w1s[:, i, j * P:(j + 1) * P], xT[:, i, :],
                            start=(i == 0), stop=(i == n_i - 1),
                        )
                    for i in range(n_i):
                        nc.tensor.matmul(
                            pB, w3s[:, i, j * P:(j + 1) * P], xT[:, i, :],
                            start=(i == 0), stop=(i == n_i - 1),
                        )
                    s1 = spool.tile([P, TB], f16)
                    nc.scalar.activation(s1, pA, AF.Silu)
                    nc.vector.tensor_mul(hT[:, j, :], pB, s1)

                # ---- mm2 -> natural-layout out block; DMA per t-chunk so
                # the last block's store overlaps its own mm2.
                o_sb = opool.tile([P, n_tc, D], f32)
                for t in range(n_tc):
                    for o in range(n_o):
                        pC = pCp.tile([P, 512], f32)
                        for j in range(n_j):
                            nc.tensor.matmul(
                                pC,
                                hT[:, j, t * P:(t + 1) * P],
                                w2s[:, j, o * 512:(o + 1) * 512],
                                start=(j == 0), stop=(j == n_j - 1),
                            )
                        nc.scalar.activation(
                            o_sb[:, t, o * 512:(o + 1) * 512], pC, AF.Copy
                        )
                    nc.scalar.dma_start(
                        out=o_r[b, :, t, :], in_=o_sb[:, t, :]
                    )

    nc.compile()
    return nc


def _get_nc(tok):
    if tok not in _CACHE:
        _CACHE[tok] = _build(tok)
    return _CACHE[tok]


def kernel(x, w1, w2, w3, m_sizes):
    from concourse.bass_utils import run_bass_kernel_spmd

    x = np.asarray(x, dtype=np.float32)
    w1 = np.asarray(w1, dtype=np.float32)
    w2 = np.asarray(w2, dtype=np.float32)
    w3 = np.asarray(w3, dtype=np.float32)
    sizes = np.asarray(m_sizes).astype(np.int64)
    offs = np.concatenate([[0], np.cumsum(sizes)])
    n_exp = sizes.shape[0]
    assert n_exp == N_CORES

    pad = int(max(int(sizes.max()), TB))
    pad = ((pad + TB - 1) // TB) * TB
    nc = _get_nc(pad)

    in_maps = []
    for e in range(N_CORES):
        xe = x[offs[e]:offs[e + 1]]
        if xe.shape[0] < pad:
            xe = np.concatenate(
                [xe, np.zeros((pad - xe.shape[0], D), dtype=np.float32)], axis=0
            )
        in_maps.append({"x": xe, "w1": w1[e], "w2": w2[e], "w3": w3[e]})

    r = run_bass_kernel_spmd(nc, in_maps, core_ids=list(range(N_CORES)))
    out = np.concatenate(
        [r.results[e]["out"][: sizes[e]] for e in range(N_CORES)], axis=0
    )
    return out.astype(np.float32)
